# revision 32
# baseline (speedup 1.0000x reference)
"""Llama-3.2 attention block (T=2048, H=2048, 32 q heads / 8 kv heads, d=64)
as a Bass/Tile kernel on 8 Trainium2 NeuronCores.

Sharding: tensor-parallel over heads. Core c owns q heads 4c..4c+3 and kv
head c (the GQA group). Each core projects its QKV shard over the full
sequence, applies RoPE, runs causal attention for its 4 heads. Attention
outputs are exchanged with one AllToAll PER HEAD (so collectives overlap
the remaining heads' compute), after which core c holds the full head
dimension for sequence chunk c. o_proj partial sums (4 contraction chunks
per head-collective) are interleaved into the attention instruction stream
and accumulated in an SBUF fp32 tile; each core writes a [256, 2048] fp32
slice of the output and the host concatenates the slices.

Layouts on device (bf16 matmul inputs, fp32 accumulation):
  - hidden and weights are pre-transposed on host so the contraction dim
    (hidden) lands on SBUF partitions.
  - QKV is produced transposed: q/k/v as [feat, seq] tiles. RoPE is applied
    in this layout: out = x * cos + (P @ x) * sin, where P is the
    rotate-half permutation done on the tensor engine.
  - q lives in q_t[m] [128, T] holding heads 2m (rows 0:64) and 2m+1
    (64:128). k lives twice: k_t (k on rows 0:64, zeros 64:128) and k2_t
    (zeros 0:64, k on 64:128). Score matmuls for even heads contract
    q_t[m] against k_t, odd heads against k2_t - the zero rows mask the
    other head, so no per-head q extraction DMAs are needed and score
    matmuls contract over K=128 (K=64 matmuls are ~2.3x slower per column).
  - scores are computed transposed (scoresT[k, q]) so softmax's exp runs on
    the scalar engine and P@V consumes probsT directly as the moving
    operand; the softmax denominator rides along as a ones-column appended
    to V. No max-subtraction is needed: |scores| <= ~20 for this problem,
    safely inside fp32 exp range. 1/denom is broadcast across partitions
    with a K=1 ones matmul on the tensor engine.
  - initial loads use >=4KB DMA descriptors split across queues and are
    issued from the Sync, Activation and GpSimd queues in parallel (per-
    descriptor cost is ~85ns regardless of size below ~4KB, and each
    dma_start costs ~0.6us of issuing-queue time).
"""

import os
import sys
import types

import numpy as np
import ml_dtypes

T = 2048
HID = 2048
NH = 32
NKV = 8
D = 64
NCORES = 8
HPC = NH // NCORES        # q heads per core = 4
FPC = HPC * D             # attention feats per core = 256
SPC = T // NCORES         # seq chunk per core after AllToAll = 256
QKV_F = FPC + 2 * D       # per-core qkv proj feats = 384
ROPE_THETA = 500000.0
SCALE = float(D) ** -0.5

_CACHE = {}


def _ensure_trace_hooks():
    """Register the NTFF profiling hook that the stub antenv package lacks."""
    if "antenv.axon_hooks" in sys.modules:
        return
    try:
        import antenv
    except ImportError:
        return
    hooks = types.ModuleType("antenv.axon_hooks")
    holder = [None]
    hooks.set_axon_ntff_profile_hook = lambda h: holder.__setitem__(0, h)
    hooks.get_axon_ntff_profile_hook = lambda: holder[0]
    antenv.axon_hooks = hooks
    sys.modules["antenv.axon_hooks"] = hooks
    try:
        from trn_agent_boot.trn_boot import _ntff_profile_via_ctypes

        hook = _ntff_profile_via_ctypes("/opt/axon/libaxon_pjrt.so")
        if hook is not None:
            hooks.set_axon_ntff_profile_hook(hook)
    except Exception:
        pass


def _build():
    from contextlib import ExitStack

    from concourse import bacc
    import concourse.mybir as mybir
    import concourse.tile as tile
    from concourse.bass import ts
    from concourse.tile import add_dep_helper

    f32 = mybir.dt.float32
    bf16 = mybir.dt.bfloat16
    AF = mybir.ActivationFunctionType
    OP = mybir.AluOpType

    KO = HID // 128           # 16 contraction chunks
    NQ = T // 512             # 4 seq chunks of 512
    NB = T // 128             # 16 k blocks of 128

    nc = bacc.Bacc("TRN2", target_bir_lowering=False, debug=False, num_devices=NCORES)

    # [128, KO, 384]: per-partition 12KB contiguous
    wT = nc.dram_tensor("wT", [128, KO, QKV_F], bf16, kind="ExternalInput")
    # [n, 128, KO, 512]: per-partition 16KB contiguous per chunk
    hT = nc.dram_tensor("hT", [T // 512, 128, HID // 128, 512], bf16, kind="ExternalInput")
    # cos/sin [n, 128, 512] f32: per-partition 2KB per chunk
    cosf = nc.dram_tensor("cosf", [NQ, 128, 512], f32, kind="ExternalInput")
    sinf = nc.dram_tensor("sinf", [NQ, 128, 512], f32, kind="ExternalInput")
    # packed constants [128, 4, 128]: perm | ident | tri | ones
    pc = nc.dram_tensor("pc", [128, 4, 128], bf16, kind="ExternalInput")
    woT = nc.dram_tensor("woT", [HID, HID], bf16, kind="ExternalInput")
    out = nc.dram_tensor("out", [SPC, HID], f32, kind="ExternalOutput")
    a2a_in = [
        nc.dram_tensor(f"a2a_in{h}", [NCORES, D, SPC], bf16) for h in range(HPC)
    ]
    a2a_out = [
        nc.dram_tensor(f"a2a_out{h}", [NCORES, D, SPC], bf16) for h in range(HPC)
    ]

    with tile.TileContext(nc) as tc, ExitStack() as ctx:
        consts = ctx.enter_context(tc.tile_pool(name="consts", bufs=1))
        persist = ctx.enter_context(tc.tile_pool(name="persist", bufs=1))

        wt_t = consts.tile([128, KO, QKV_F], bf16, tag="wt")
        ht0_t = consts.tile([128, KO, 512], bf16, tag="ht0")
        cos_t = consts.tile([128, NQ, 512], f32, tag="cos")
        sin_t = consts.tile([128, NQ, 512], f32, tag="sin")
        pc_t = consts.tile([128, 4, 128], bf16, tag="pc")
        perm_t = pc_t[:, 0, :]
        ident_t = pc_t[:, 1, :]
        tri_t = pc_t[:, 2, :]
        ones_t = pc_t[:, 3, :]
        wo_t = consts.tile([128, KO, HID], bf16, tag="wo")

        # Startup loads. Per-DMA-queue bandwidth is ~22.5GB/s and descriptor
        # rate ~85ns each, so keep descriptors ~1-4KB and spread transfers
        # over many queues. Issue from the two HWDGE queues (sync + scalar,
        # ~0.6us fixed per issue); the gpsimd queue generates descriptors in
        # SOFTWARE (~35ns/descriptor of queue time) so it carries ONLY the
        # collectives. k-group 0 of wt and ht0 goes out first so the first
        # QKV psum group starts ASAP.
        # Early alignment collective: absorb SPMD launch skew while the PE
        # is busy with QKV so attention-phase collectives don't pay it.
        align_in = nc.dram_tensor("align_in", [NCORES, 1, 1], bf16)
        align_out = nc.dram_tensor("align_out", [NCORES, 1, 1], bf16)
        nc.gpsimd.collective_compute(
            "AllToAll",
            mybir.AluOpType.bypass,
            replica_groups=[list(range(NCORES))],
            ins=[align_in.ap()],
            outs=[align_out.ap()],
        )
        for p4 in range(4):
            nc.sync.dma_start(wt_t[ts(p4, 32), 0:4], wT.ap()[ts(p4, 32), 0:4])
            nc.scalar.dma_start(ht0_t[ts(p4, 32), 0:4], hT.ap()[0, ts(p4, 32), 0:4])
        for p2 in range(2):
            nc.sync.dma_start(cos_t[ts(p2, 64), 0, :], cosf.ap()[0, ts(p2, 64)])
            nc.sync.dma_start(sin_t[ts(p2, 64), 0, :], sinf.ap()[0, ts(p2, 64)])
        for g in range(1, 4):
            for p2 in range(2):
                nc.sync.dma_start(
                    wt_t[ts(p2, 64), ts(g, 4)], wT.ap()[ts(p2, 64), ts(g, 4)]
                )
            for p4 in range(4):
                nc.scalar.dma_start(
                    ht0_t[ts(p4, 32), ts(g, 4)], hT.ap()[0, ts(p4, 32), ts(g, 4)]
                )
        for p2 in range(2):
            nc.sync.dma_start(pc_t[ts(p2, 64)], pc.ap()[ts(p2, 64)])
        for n in range(1, NQ):
            nc.sync.dma_start(cos_t[:, n, :], cosf.ap()[n])
            nc.sync.dma_start(sin_t[:, n, :], sinf.ap()[n])

        # Persistent activation tiles (live across phases A/B).
        q_t = [persist.tile([128, T], bf16, tag=f"q{p}", name=f"q{p}") for p in range(2)]
        k_t = persist.tile([128, T], bf16, tag="kt")
        k2_t = persist.tile([128, T], bf16, tag="k2t")
        vaug_t = persist.tile([128, KO, D + 1], bf16, tag="vaug")
        o_acc = persist.tile([128, 8, 512], f32, tag="oacc")

        # zero the K-padding rows once
        nc.vector.memset(k_t[64:128, :], 0.0)
        nc.vector.memset(k2_t[0:64, :], 0.0)

        # Shared PSUM pools for the whole kernel (no pool-scope drain
        # barrier between phases): ps1 2x2 banks (QKV psum / score pairs),
        # ps2 2x1 (rope shift / PV accum), ps3 2x1 (v transpose / o_proj
        # partials / denom broadcast).
        ps1 = ctx.enter_context(tc.tile_pool(name="ps1", bufs=2, space="PSUM"))
        ps2 = ctx.enter_context(tc.tile_pool(name="ps2", bufs=2, space="PSUM"))
        ps3 = ctx.enter_context(tc.tile_pool(name="ps3", bufs=2, space="PSUM"))

        # ---- Phase A: QKV projection + RoPE (outputs transposed [feat, seq]) ----
        with nc.named_scope("qkv"):
            with (
                tc.tile_pool(name="htp", bufs=2) as ht_pool,
                tc.tile_pool(name="atmp", bufs=3) as atmp,
            ):
                psA = psAsh = psV = None  # replaced by shared pools
                nc.vector.memset(vaug_t[:, :, D:D + 1], 1.0)
                for n in range(NQ):
                    if n == 0:
                        ht_t = ht0_t
                    else:
                        ht_t = ht_pool.tile([128, KO, 512], bf16, tag="ht")
                        gate = None
                        for k4 in range(4):
                            dh = nc.sync.dma_start(
                                ht_t[:, ts(k4, 4), :], hT.ap()[n, :, ts(k4, 4), :]
                            )
                            gate = gate or dh

                    pending = None
                    for m in range(3):
                        pq = ps1.tile([128, 2, 512], f32, tag="ps1", name="pq")[:, 0, :]
                        for k in range(KO):
                            nc.tensor.matmul(
                                pq,
                                wt_t[:, k, ts(m, 128)],
                                ht_t[:, k, :],
                                start=(k == 0),
                                stop=(k == KO - 1),
                            )
                        xb = atmp.tile([128, 512], bf16, tag="xb")
                        nc.vector.tensor_copy(xb, pq)
                        if pending is not None:
                            pending()
                            pending = None
                        if m < 2:
                            def rope_q(m=m, n=n, xb=xb):
                                # two q heads: rotate-half via PE perm + DVE
                                psh = ps2.tile([128, 512], f32, tag="ps2", name="psh")
                                nc.tensor.matmul(psh, perm_t, xb, start=True, stop=True)
                                t1 = atmp.tile([128, 512], f32, tag="t1", name="t1")
                                nc.vector.tensor_tensor(t1, xb, cos_t[:, n, :], OP.mult)
                                t2 = atmp.tile([128, 512], f32, tag="t2", name="t2")
                                nc.vector.tensor_tensor(t2, psh, sin_t[:, n, :], OP.mult)
                                nc.vector.tensor_tensor(q_t[m][:, ts(n, 512)], t1, t2, OP.add)
                            pending = rope_q
                        else:
                            def rope_kv(n=n, xb=xb):
                                # k head on partitions 0:64 (rope), v on 64:128
                                psh = ps2.tile([128, 512], f32, tag="ps2", name="psh")
                                nc.tensor.matmul(
                                    psh[0:64, :], perm_t[0:64, 0:64], xb[0:64, :],
                                    start=True, stop=True,
                                )
                                t1 = atmp.tile([128, 512], f32, tag="t1", name="t1")
                                nc.vector.tensor_tensor(
                                    t1[0:64, :], xb[0:64, :], cos_t[0:64, n, :], OP.mult
                                )
                                t2 = atmp.tile([128, 512], f32, tag="t2", name="t2")
                                nc.vector.tensor_tensor(
                                    t2[0:64, :], psh[0:64, :], sin_t[0:64, n, :], OP.mult
                                )
                                nc.vector.tensor_tensor(
                                    k_t[0:64, ts(n, 512)], t1[0:64, :], t2[0:64, :], OP.add
                                )
                                # odd heads contract against k at partitions 64:128
                                nc.sync.dma_start(
                                    k2_t[64:128, ts(n, 512)], k_t[0:64, ts(n, 512)]
                                )
                                # v transpose straight from xb's 64:128 rows
                                for j in range(4 * n, 4 * n + 4):
                                    pv = ps3.tile([128, 1024], bf16, tag="ps3", name="pv")[:, 0:D]
                                    nc.tensor.transpose(
                                        pv,
                                        xb[64:128, ts(j - 4 * n, 128)],
                                        ident_t[64:128, 64:128],
                                        tile_position=(64, 0),
                                    )
                                    nc.vector.tensor_copy(vaug_t[:, j, 0:D], pv)
                            pending = rope_kv
                    if pending is not None:
                        pending()

        # ---- Phase B: causal attention + per-head AllToAll + o_proj ----
        a2a_dmas = {h: [] for h in range(HPC)}
        ccs = {}

        def _emit_cc(h):
            cc = nc.gpsimd.collective_compute(
                "AllToAll",
                OP.bypass,
                replica_groups=[list(range(NCORES))],
                ins=[a2a_in[h].ap()],
                outs=[a2a_out[h].ap()],
            )
            for dd in a2a_dmas[h]:
                add_dep_helper(cc.ins, dd.ins, sync=True, reason="cc waits a2a stage-in")
            ccs[h] = cc
            return cc

        with nc.named_scope("attn"):
            with (
                tc.tile_pool(name="probs", bufs=2) as probs_pool,
                tc.tile_pool(name="btmp", bufs=6) as btmp,
                tc.tile_pool(name="lo", bufs=2) as lo_pool,
                tc.tile_pool(name="dtmp", bufs=2) as dtmp,
            ):
                psS, psO, psD = ps1, ps2, ps3
                lo_t = {}

                def emit_oproj_load(g, dual=False):
                    # stage a2a_out[g] (contraction rows for o_proj group g).
                    # Gated on cc_g; emitted with enough emission-order slack
                    # that the sync queue rarely blocks on the cc semaphore.
                    lo_t[g] = lo_pool.tile([128, 4, SPC], bf16, tag="lo", name="lo")
                    flat = a2a_out[g].ap().rearrange("a f s -> (a f) s")
                    for j in range(4):
                        for p2 in range(2):
                            eng = nc.scalar if (dual and p2 == 1) else nc.sync
                            dl = eng.dma_start(
                                lo_t[g][ts(p2, 64), j, :],
                                flat[j * 128 + p2 * 64:j * 128 + (p2 + 1) * 64, :],
                            )
                            add_dep_helper(
                                dl.ins, ccs[g].ins, sync=True, reason="o_proj waits cc"
                            )

                def emit_oproj_tile(g, t, alt=False):
                    # t = m * 4 + e4; accumulate 4 contraction chunks into o_acc
                    m, e4 = t // 4, t % 4
                    if alt:
                        # final group: attention psum pools are free, rotate
                        # across all three so adds never stall the PE
                        pool, ptag = [(psD, "ps3"), (psS, "ps1"), (psO, "ps2")][t % 3]
                    else:
                        pool, ptag = psD, "ps3"
                    pso = pool.tile([128, 2, 512] if ptag == "ps1" else [128, 512],
                                    f32, tag=ptag, name="pso")
                    if ptag == "ps1":
                        pso = pso[:, 0, :]
                    for j in range(4):
                        nc.tensor.matmul(
                            pso,
                            lo_t[g][:, j, ts(m, 128)],
                            wo_t[:, 4 * g + j, ts(e4, 512)],
                            start=(j == 0),
                            stop=(j == 3),
                        )
                    if g == 0:
                        nc.vector.tensor_copy(o_acc[:, t, :], pso)
                    elif g < HPC - 1:
                        nc.vector.tensor_tensor(o_acc[:, t, :], pso, o_acc[:, t, :], OP.add)
                    else:
                        ob = dtmp.tile([128, 512], f32, tag="ob", name="ob")
                        nc.vector.tensor_tensor(ob, pso, o_acc[:, t, :], OP.add)
                        for p4 in range(4):
                            eng = nc.sync if p4 < 2 else nc.scalar
                            eng.dma_start(
                                out.ap()[m * 128 + p4 * 32:m * 128 + (p4 + 1) * 32,
                                         ts(e4, 512)],
                                ob[ts(p4, 32)],
                            )

                # o_proj work units [group, tile-range] pop into the PE stream
                # at i-chunk boundaries once their collective's data is safely
                # landed (cc + lo stage-in take ~13us after head g ends)
                oproj_units = []
                pending_norm = [None]

                def flush_norm():
                    if pending_norm[0] is not None:
                        pending_norm[0]()
                        pending_norm[0] = None

                def pop_units(k):
                    for _ in range(k):
                        if oproj_units:
                            g, t0, t1 = oproj_units.pop(0)
                            for t in range(t0, t1):
                                emit_oproj_tile(g, t)

                for h in range(HPC):
                    kt_h = k_t if h % 2 == 0 else k2_t
                    qm = q_t[h // 2]
                    if h in (0, 1):
                        # bulk w_o stream split across heads 0/1: halves the
                        # queue flood so cc0's staging and lo0 land in time
                        # for group 0's first pop; the scalar queue stays
                        # clear for exp
                        for k in range(8 * h, 8 * h + 8):
                            nc.sync.dma_start(wo_t[:, k, :], woT.ap()[ts(k, 128), :])
                    for i in range(NQ):
                        nj = 4 * i + 4
                        pr = probs_pool.tile([128, NB, 512], bf16, tag="pr")
                        po = psO.tile([128, 512], f32, tag="ps2", name="po")[0:D + 1, :]
                        # chunk j-blocks (pairs of full-width blocks share one
                        # psum tile + exp call), then emit P@V lagged a few
                        # chunks behind the scores so PE never stalls on exp
                        chunks = []
                        j = 0
                        while j < nj:
                            if j - 4 * i < -1:
                                chunks.append([j, j + 1])
                                j += 2
                            else:
                                chunks.append([j])
                                j += 1

                        def emit_scores(js, kt_h=kt_h, qm=qm, i=i, pr=pr):
                            if len(js) == 2:
                                pss = psS.tile([128, 2, 512], f32, tag="ps1", name="pss")
                                for u in range(2):
                                    nc.tensor.matmul(
                                        pss[:, u, :],
                                        kt_h[:, ts(js[u], 128)],
                                        qm[:, ts(i, 512)],
                                        start=True, stop=True,
                                    )
                                nc.scalar.activation(
                                    pr[:, js[0]:js[0] + 2, :], pss, AF.Exp, scale=SCALE
                                )
                            else:
                                jj = js[0]
                                r = jj - 4 * i
                                off = max(0, r) * 128
                                pss = psS.tile([128, 2, 512], f32, tag="ps1", name="pss")
                                nc.tensor.matmul(
                                    pss[:, 0, off:512],
                                    kt_h[:, ts(jj, 128)],
                                    qm[:, i * 512 + off:(i + 1) * 512],
                                    start=True, stop=True,
                                )
                                nc.scalar.activation(
                                    pr[:, jj, off:512], pss[:, 0, off:512],
                                    AF.Exp, scale=SCALE,
                                )
                                if r >= 0:  # block overlapping the causal diagonal
                                    nc.vector.tensor_tensor(
                                        pr[:, jj, off:off + 128],
                                        pr[:, jj, off:off + 128],
                                        tri_t, OP.mult,
                                    )

                        def emit_pv(js, i=i, pr=pr, po=po, nj=nj):
                            for jj in js:
                                off = max(0, jj - 4 * i) * 128
                                nc.tensor.matmul(
                                    po[:, off:512], vaug_t[:, jj, :], pr[:, jj, off:512],
                                    start=(jj == 0), stop=(jj == nj - 1),
                                )

                        LAG = 3
                        for ci, ch in enumerate(chunks):
                            emit_scores(ch)
                            if ci == min(1, len(chunks) - 1):
                                # previous i's normalize chain lands here so
                                # its PE step never waits on the DVE chain
                                flush_norm()
                                if i in (0, 1, 3):
                                    pop_units(1)
                            if ci >= LAG:
                                emit_pv(chunks[ci - LAG])
                        for ci in range(max(0, len(chunks) - LAG), len(chunks)):
                            emit_pv(chunks[ci])

                        def norm(h=h, i=i, po=po):
                            # oT[f, q] = po[f, q] / den[q]; den row broadcast
                            # across partitions via a K=1 ones matmul + 1/x on DVE
                            dbc = btmp.tile([D + 1, 512], bf16, tag="dbc", name="dbc")
                            nc.vector.tensor_copy(dbc[D:D + 1, :], po[D:D + 1, :])
                            pb = psD.tile([128, 512], f32, tag="ps3", name="pb")
                            nc.tensor.matmul(
                                pb[0:D, :], ones_t[D:D + 1, 0:D], dbc[D:D + 1, :],
                                start=True, stop=True,
                            )
                            rbs = btmp.tile([D, 512], f32, tag="rbs", name="rbs")
                            nc.vector.reciprocal_approx_fast(out=rbs, in_=pb[0:D, :])
                            oth = btmp.tile([D, 512], bf16, tag="oth", name="oth")
                            nc.vector.tensor_tensor(oth, po[0:D, :], rbs, OP.mult)
                            for half in range(2):
                                dd = nc.sync.dma_start(
                                    a2a_in[h].ap()[2 * i + half, :, :],
                                    oth[:, ts(half, 256)],
                                )
                                a2a_dmas[h].append(dd)
                        pending_norm[0] = norm
                        if i == 2 and h in (1, 2):
                            emit_oproj_load(h - 1)
                        if i == 1 and h == 3:
                            emit_oproj_load(2)
                        if i == 2 and h == 3:
                            oproj_units.extend([(2, 0, 4), (2, 4, 8)])
                    flush_norm()
                    _emit_cc(h)
                    # previous head's o_proj: 4 tiles now (PE covers the
                    # freshly-emitted collective), 2+2 early next head
                    if 1 <= h < 3:
                        oproj_units.extend(
                            [(h - 1, 0, 4), (h - 1, 4, 6), (h - 1, 6, 8)]
                        )
                    if h >= 2:
                        pop_units(1)

                # final group: head 3 + output assembly, double-buffered
                # across two psum pools so the adds never stall the PE
                g = HPC - 1
                emit_oproj_load(g, dual=True)
                pop_units(len(oproj_units))
                for t in range(8):
                    emit_oproj_tile(g, t, alt=True)

    nc.compile()
    return nc


def _get_nc():
    if "nc" not in _CACHE:
        _CACHE["nc"] = _build()
    return _CACHE["nc"]


def _host_prep(hidden_states, positions, w_qkv, w_o):
    bf16 = ml_dtypes.bfloat16
    hTb = np.ascontiguousarray(hidden_states.astype(np.float32).T).astype(bf16)
    # pretile to [n, p, ko, s] so each chunk is contiguous per partition
    hTt = np.ascontiguousarray(
        hTb.reshape(HID // 128, 128, T // 512, 512).transpose(2, 1, 0, 3)
    )
    woTb = np.ascontiguousarray(w_o.astype(np.float32).T).astype(bf16)
    # o_proj contraction order matches the per-head AllToAll staging:
    # group h holds head h of every source core (src-major within group)
    srcs = np.arange(NCORES)[:, None]
    dd_ = np.arange(D)[None, :]
    idx = np.concatenate(
        [(srcs * FPC + h * D + dd_).reshape(-1) for h in range(HPC)]
    )
    woTb = np.ascontiguousarray(woTb[idx])

    inv = 1.0 / (ROPE_THETA ** (np.arange(0, D, 2, dtype=np.float32) / D))  # [32]
    ang = positions.astype(np.float32)[:, None] * inv[None, :]              # [T, 32]
    cos = np.cos(ang).T  # [32, T]
    sin = np.sin(ang).T
    p = np.arange(128)
    fr = (p % D) % (D // 2)
    sgn = np.where((p % D) < (D // 2), -1.0, 1.0).astype(np.float32)
    cosf = np.ascontiguousarray(cos[fr])                     # [128, T]
    sinf = np.ascontiguousarray(sin[fr] * sgn[:, None])      # [128, T]
    # [n, 128, 512] chunk layout
    cosf = np.ascontiguousarray(cosf.reshape(128, NQ := T // 512, 512).transpose(1, 0, 2))
    sinf = np.ascontiguousarray(sinf.reshape(128, NQ, 512).transpose(1, 0, 2))

    partner = np.where((p % D) < (D // 2), p + D // 2, p - D // 2)
    perm = np.zeros((128, 128), dtype=np.float32)
    perm[p, partner] = 1.0
    ident = np.eye(128, dtype=np.float32)
    tri = (np.arange(128)[None, :] >= np.arange(128)[:, None]).astype(np.float32)
    ones_m = np.ones((128, 128), dtype=np.float32)
    pcb = np.ascontiguousarray(
        np.stack([perm, ident, tri, ones_m], axis=1).astype(bf16)
    )  # [128, 4, 128]

    q_size = NH * D
    kv_size = NKV * D
    in_maps = []
    for c in range(NCORES):
        wq = w_qkv[c * FPC:(c + 1) * FPC]
        wk = w_qkv[q_size + c * D:q_size + (c + 1) * D]
        wv = w_qkv[q_size + kv_size + c * D:q_size + kv_size + (c + 1) * D]
        wTc = np.ascontiguousarray(
            np.concatenate([wq, wk, wv], axis=0).astype(np.float32).T
        ).astype(bf16)
        # [128, KO, 384]: per-partition contiguous across contraction chunks
        wTc = np.ascontiguousarray(
            wTc.reshape(HID // 128, 128, QKV_F).transpose(1, 0, 2)
        )
        in_maps.append(
            {
                "hT": hTt,
                "wT": wTc,
                "cosf": cosf,
                "sinf": sinf,
                "pc": pcb,
                "woT": woTb,
            }
        )
    return in_maps


def run(inputs, trace=False):
    """Run on 8 NeuronCores; returns (full_output, BassKernelResults)."""
    if trace:
        _ensure_trace_hooks()
    from concourse import bass_utils

    if trace:
        bass_utils.upload_artifacts = lambda tmpdir: tmpdir
    nc = _get_nc()
    in_maps = _host_prep(
        np.asarray(inputs["hidden_states"]),
        np.asarray(inputs["positions"]),
        np.asarray(inputs["w_qkv"]),
        np.asarray(inputs["w_o"]),
    )
    res = bass_utils.run_bass_kernel_spmd(
        nc, in_maps, core_ids=list(range(NCORES)), trace=trace
    )
    full = np.concatenate(
        [res.results[c]["out"] for c in range(NCORES)], axis=0
    ).astype(np.float32)
    return full, res


def kernel(**inputs) -> np.ndarray:
    trace = bool(os.environ.get("KERNEL_TRACE"))
    full, _ = run(inputs, trace=trace)
    return full


# revision 33
# speedup vs baseline: 1.0018x; 1.0018x over previous
"""Llama-3.2 attention block (T=2048, H=2048, 32 q heads / 8 kv heads, d=64)
as a Bass/Tile kernel on 8 Trainium2 NeuronCores.

Sharding: tensor-parallel over heads. Core c owns q heads 4c..4c+3 and kv
head c (the GQA group). Each core projects its QKV shard over the full
sequence, applies RoPE, runs causal attention for its 4 heads. Attention
outputs are exchanged with one AllToAll PER HEAD (so collectives overlap
the remaining heads' compute), after which core c holds the full head
dimension for sequence chunk c. o_proj partial sums (4 contraction chunks
per head-collective) are interleaved into the attention instruction stream
and accumulated in an SBUF fp32 tile; each core writes a [256, 2048] fp32
slice of the output and the host concatenates the slices.

Layouts on device (bf16 matmul inputs, fp32 accumulation):
  - hidden and weights are pre-transposed on host so the contraction dim
    (hidden) lands on SBUF partitions.
  - QKV is produced transposed: q/k/v as [feat, seq] tiles. RoPE is applied
    in this layout: out = x * cos + (P @ x) * sin, where P is the
    rotate-half permutation done on the tensor engine.
  - q lives in q_t[m] [128, T] holding heads 2m (rows 0:64) and 2m+1
    (64:128). k lives twice: k_t (k on rows 0:64, zeros 64:128) and k2_t
    (zeros 0:64, k on 64:128). Score matmuls for even heads contract
    q_t[m] against k_t, odd heads against k2_t - the zero rows mask the
    other head, so no per-head q extraction DMAs are needed and score
    matmuls contract over K=128 (K=64 matmuls are ~2.3x slower per column).
  - scores are computed transposed (scoresT[k, q]) so softmax's exp runs on
    the scalar engine and P@V consumes probsT directly as the moving
    operand; the softmax denominator rides along as a ones-column appended
    to V. No max-subtraction is needed: |scores| <= ~20 for this problem,
    safely inside fp32 exp range. 1/denom is broadcast across partitions
    with a K=1 ones matmul on the tensor engine.
  - initial loads use >=4KB DMA descriptors split across queues and are
    issued from the Sync, Activation and GpSimd queues in parallel (per-
    descriptor cost is ~85ns regardless of size below ~4KB, and each
    dma_start costs ~0.6us of issuing-queue time).
"""

import os
import sys
import types

import numpy as np
import ml_dtypes

T = 2048
HID = 2048
NH = 32
NKV = 8
D = 64
NCORES = 8
HPC = NH // NCORES        # q heads per core = 4
FPC = HPC * D             # attention feats per core = 256
SPC = T // NCORES         # seq chunk per core after AllToAll = 256
QKV_F = FPC + 2 * D       # per-core qkv proj feats = 384
ROPE_THETA = 500000.0
SCALE = float(D) ** -0.5

_CACHE = {}


def _ensure_trace_hooks():
    """Register the NTFF profiling hook that the stub antenv package lacks."""
    if "antenv.axon_hooks" in sys.modules:
        return
    try:
        import antenv
    except ImportError:
        return
    hooks = types.ModuleType("antenv.axon_hooks")
    holder = [None]
    hooks.set_axon_ntff_profile_hook = lambda h: holder.__setitem__(0, h)
    hooks.get_axon_ntff_profile_hook = lambda: holder[0]
    antenv.axon_hooks = hooks
    sys.modules["antenv.axon_hooks"] = hooks
    try:
        from trn_agent_boot.trn_boot import _ntff_profile_via_ctypes

        hook = _ntff_profile_via_ctypes("/opt/axon/libaxon_pjrt.so")
        if hook is not None:
            hooks.set_axon_ntff_profile_hook(hook)
    except Exception:
        pass


def _build():
    from contextlib import ExitStack

    from concourse import bacc
    import concourse.mybir as mybir
    import concourse.tile as tile
    from concourse.bass import ts
    from concourse.tile import add_dep_helper

    f32 = mybir.dt.float32
    bf16 = mybir.dt.bfloat16
    AF = mybir.ActivationFunctionType
    OP = mybir.AluOpType

    KO = HID // 128           # 16 contraction chunks
    NQ = T // 512             # 4 seq chunks of 512
    NB = T // 128             # 16 k blocks of 128

    nc = bacc.Bacc("TRN2", target_bir_lowering=False, debug=False, num_devices=NCORES)

    # [128, KO, 384]: per-partition 12KB contiguous
    wT = nc.dram_tensor("wT", [128, KO, QKV_F], bf16, kind="ExternalInput")
    # [n, 128, KO, 512]: per-partition 16KB contiguous per chunk
    hT = nc.dram_tensor("hT", [T // 512, 128, HID // 128, 512], bf16, kind="ExternalInput")
    # cos/sin [n, 128, 512] f32: per-partition 2KB per chunk
    cosf = nc.dram_tensor("cosf", [NQ, 128, 512], f32, kind="ExternalInput")
    sinf = nc.dram_tensor("sinf", [NQ, 128, 512], f32, kind="ExternalInput")
    # packed constants [128, 4, 128]: perm | ident | tri | ones
    pc = nc.dram_tensor("pc", [128, 4, 128], bf16, kind="ExternalInput")
    woT = nc.dram_tensor("woT", [HID, HID], bf16, kind="ExternalInput")
    out = nc.dram_tensor("out", [SPC, HID], f32, kind="ExternalOutput")
    a2a_in = [
        nc.dram_tensor(f"a2a_in{h}", [NCORES, D, SPC], bf16) for h in range(HPC)
    ]
    a2a_out = [
        nc.dram_tensor(f"a2a_out{h}", [NCORES, D, SPC], bf16) for h in range(HPC)
    ]

    with tile.TileContext(nc) as tc, ExitStack() as ctx:
        consts = ctx.enter_context(tc.tile_pool(name="consts", bufs=1))
        persist = ctx.enter_context(tc.tile_pool(name="persist", bufs=1))

        wt_t = consts.tile([128, KO, QKV_F], bf16, tag="wt")
        ht0_t = consts.tile([128, KO, 512], bf16, tag="ht0")
        cos_t = consts.tile([128, NQ, 512], f32, tag="cos")
        sin_t = consts.tile([128, NQ, 512], f32, tag="sin")
        pc_t = consts.tile([128, 4, 128], bf16, tag="pc")
        perm_t = pc_t[:, 0, :]
        ident_t = pc_t[:, 1, :]
        tri_t = pc_t[:, 2, :]
        ones_t = pc_t[:, 3, :]
        wo_t = consts.tile([128, KO, HID], bf16, tag="wo")

        # Startup loads. Per-DMA-queue bandwidth is ~22.5GB/s and descriptor
        # rate ~85ns each, so keep descriptors ~1-4KB and spread transfers
        # over many queues. Issue from the two HWDGE queues (sync + scalar,
        # ~0.6us fixed per issue); the gpsimd queue generates descriptors in
        # SOFTWARE (~35ns/descriptor of queue time) so it carries ONLY the
        # collectives. k-group 0 of wt and ht0 goes out first so the first
        # QKV psum group starts ASAP.
        # Early alignment collective: absorb SPMD launch skew while the PE
        # is busy with QKV so attention-phase collectives don't pay it.
        align_in = nc.dram_tensor("align_in", [NCORES, 1, 1], bf16)
        align_out = nc.dram_tensor("align_out", [NCORES, 1, 1], bf16)
        nc.gpsimd.collective_compute(
            "AllToAll",
            mybir.AluOpType.bypass,
            replica_groups=[list(range(NCORES))],
            ins=[align_in.ap()],
            outs=[align_out.ap()],
        )
        for p4 in range(4):
            nc.sync.dma_start(wt_t[ts(p4, 32), 0:4], wT.ap()[ts(p4, 32), 0:4])
            nc.scalar.dma_start(ht0_t[ts(p4, 32), 0:4], hT.ap()[0, ts(p4, 32), 0:4])
        for p2 in range(2):
            nc.sync.dma_start(cos_t[ts(p2, 64), 0, :], cosf.ap()[0, ts(p2, 64)])
            nc.sync.dma_start(sin_t[ts(p2, 64), 0, :], sinf.ap()[0, ts(p2, 64)])
        for g in range(1, 4):
            for p2 in range(2):
                nc.sync.dma_start(
                    wt_t[ts(p2, 64), ts(g, 4)], wT.ap()[ts(p2, 64), ts(g, 4)]
                )
            for p2 in range(2):
                nc.scalar.dma_start(
                    ht0_t[ts(p2, 64), ts(g, 4)], hT.ap()[0, ts(p2, 64), ts(g, 4)]
                )
        for p2 in range(2):
            nc.sync.dma_start(pc_t[ts(p2, 64)], pc.ap()[ts(p2, 64)])
        for n in range(1, NQ):
            nc.sync.dma_start(cos_t[:, n, :], cosf.ap()[n])
            nc.sync.dma_start(sin_t[:, n, :], sinf.ap()[n])

        # Persistent activation tiles (live across phases A/B).
        q_t = [persist.tile([128, T], bf16, tag=f"q{p}", name=f"q{p}") for p in range(2)]
        k_t = persist.tile([128, T], bf16, tag="kt")
        k2_t = persist.tile([128, T], bf16, tag="k2t")
        vaug_t = persist.tile([128, KO, D + 1], bf16, tag="vaug")
        o_acc = persist.tile([128, 8, 512], f32, tag="oacc")

        # zero the K-padding rows once
        nc.vector.memset(k_t[64:128, :], 0.0)
        nc.vector.memset(k2_t[0:64, :], 0.0)

        # Shared PSUM pools for the whole kernel (no pool-scope drain
        # barrier between phases): ps1 2x2 banks (QKV psum / score pairs),
        # ps2 2x1 (rope shift / PV accum), ps3 2x1 (v transpose / o_proj
        # partials / denom broadcast).
        ps1 = ctx.enter_context(tc.tile_pool(name="ps1", bufs=2, space="PSUM"))
        ps2 = ctx.enter_context(tc.tile_pool(name="ps2", bufs=2, space="PSUM"))
        ps3 = ctx.enter_context(tc.tile_pool(name="ps3", bufs=2, space="PSUM"))

        # ---- Phase A: QKV projection + RoPE (outputs transposed [feat, seq]) ----
        with nc.named_scope("qkv"):
            with (
                tc.tile_pool(name="htp", bufs=2) as ht_pool,
                tc.tile_pool(name="atmp", bufs=3) as atmp,
            ):
                psA = psAsh = psV = None  # replaced by shared pools
                nc.vector.memset(vaug_t[:, :, D:D + 1], 1.0)
                for n in range(NQ):
                    if n == 0:
                        ht_t = ht0_t
                    else:
                        ht_t = ht_pool.tile([128, KO, 512], bf16, tag="ht")
                        gate = None
                        for k4 in range(4):
                            dh = nc.sync.dma_start(
                                ht_t[:, ts(k4, 4), :], hT.ap()[n, :, ts(k4, 4), :]
                            )
                            gate = gate or dh

                    pending = None
                    for m in range(3):
                        pq = ps1.tile([128, 2, 512], f32, tag="ps1", name="pq")[:, 0, :]
                        for k in range(KO):
                            nc.tensor.matmul(
                                pq,
                                wt_t[:, k, ts(m, 128)],
                                ht_t[:, k, :],
                                start=(k == 0),
                                stop=(k == KO - 1),
                            )
                        xb = atmp.tile([128, 512], bf16, tag="xb")
                        nc.vector.tensor_copy(xb, pq)
                        if pending is not None:
                            pending()
                            pending = None
                        if m < 2:
                            def rope_q(m=m, n=n, xb=xb):
                                # two q heads: rotate-half via PE perm + DVE
                                psh = ps2.tile([128, 512], f32, tag="ps2", name="psh")
                                nc.tensor.matmul(psh, perm_t, xb, start=True, stop=True)
                                t1 = atmp.tile([128, 512], f32, tag="t1", name="t1")
                                nc.vector.tensor_tensor(t1, xb, cos_t[:, n, :], OP.mult)
                                t2 = atmp.tile([128, 512], f32, tag="t2", name="t2")
                                nc.vector.tensor_tensor(t2, psh, sin_t[:, n, :], OP.mult)
                                nc.vector.tensor_tensor(q_t[m][:, ts(n, 512)], t1, t2, OP.add)
                            pending = rope_q
                        else:
                            def rope_kv(n=n, xb=xb):
                                # k head on partitions 0:64 (rope), v on 64:128
                                psh = ps2.tile([128, 512], f32, tag="ps2", name="psh")
                                nc.tensor.matmul(
                                    psh[0:64, :], perm_t[0:64, 0:64], xb[0:64, :],
                                    start=True, stop=True,
                                )
                                t1 = atmp.tile([128, 512], f32, tag="t1", name="t1")
                                nc.vector.tensor_tensor(
                                    t1[0:64, :], xb[0:64, :], cos_t[0:64, n, :], OP.mult
                                )
                                t2 = atmp.tile([128, 512], f32, tag="t2", name="t2")
                                nc.vector.tensor_tensor(
                                    t2[0:64, :], psh[0:64, :], sin_t[0:64, n, :], OP.mult
                                )
                                nc.vector.tensor_tensor(
                                    k_t[0:64, ts(n, 512)], t1[0:64, :], t2[0:64, :], OP.add
                                )
                                # odd heads contract against k at partitions 64:128
                                nc.sync.dma_start(
                                    k2_t[64:128, ts(n, 512)], k_t[0:64, ts(n, 512)]
                                )
                                # v transpose straight from xb's 64:128 rows
                                for j in range(4 * n, 4 * n + 4):
                                    pv = ps3.tile([128, 1024], bf16, tag="ps3", name="pv")[:, 0:D]
                                    nc.tensor.transpose(
                                        pv,
                                        xb[64:128, ts(j - 4 * n, 128)],
                                        ident_t[64:128, 64:128],
                                        tile_position=(64, 0),
                                    )
                                    nc.vector.tensor_copy(vaug_t[:, j, 0:D], pv)
                            pending = rope_kv
                    if pending is not None:
                        pending()

        # ---- Phase B: causal attention + per-head AllToAll + o_proj ----
        a2a_dmas = {h: [] for h in range(HPC)}
        ccs = {}

        def _emit_cc(h):
            cc = nc.gpsimd.collective_compute(
                "AllToAll",
                OP.bypass,
                replica_groups=[list(range(NCORES))],
                ins=[a2a_in[h].ap()],
                outs=[a2a_out[h].ap()],
            )
            for dd in a2a_dmas[h]:
                add_dep_helper(cc.ins, dd.ins, sync=True, reason="cc waits a2a stage-in")
            ccs[h] = cc
            return cc

        with nc.named_scope("attn"):
            with (
                tc.tile_pool(name="probs", bufs=2) as probs_pool,
                tc.tile_pool(name="btmp", bufs=6) as btmp,
                tc.tile_pool(name="lo", bufs=2) as lo_pool,
                tc.tile_pool(name="dtmp", bufs=2) as dtmp,
            ):
                psS, psO, psD = ps1, ps2, ps3
                lo_t = {}

                def emit_oproj_load(g, dual=False):
                    # stage a2a_out[g] (contraction rows for o_proj group g).
                    # Gated on cc_g; emitted with enough emission-order slack
                    # that the sync queue rarely blocks on the cc semaphore.
                    lo_t[g] = lo_pool.tile([128, 4, SPC], bf16, tag="lo", name="lo")
                    flat = a2a_out[g].ap().rearrange("a f s -> (a f) s")
                    for j in range(4):
                        for p2 in range(2):
                            eng = nc.scalar if (dual and p2 == 1) else nc.sync
                            dl = eng.dma_start(
                                lo_t[g][ts(p2, 64), j, :],
                                flat[j * 128 + p2 * 64:j * 128 + (p2 + 1) * 64, :],
                            )
                            add_dep_helper(
                                dl.ins, ccs[g].ins, sync=True, reason="o_proj waits cc"
                            )

                def emit_oproj_tile(g, t, alt=False):
                    # t = m * 4 + e4; accumulate 4 contraction chunks into o_acc
                    m, e4 = t // 4, t % 4
                    if alt:
                        # final group: attention psum pools are free, rotate
                        # across all three so adds never stall the PE
                        pool, ptag = [(psD, "ps3"), (psS, "ps1"), (psO, "ps2")][t % 3]
                    else:
                        pool, ptag = psD, "ps3"
                    pso = pool.tile([128, 2, 512] if ptag == "ps1" else [128, 512],
                                    f32, tag=ptag, name="pso")
                    if ptag == "ps1":
                        pso = pso[:, 0, :]
                    for j in range(4):
                        nc.tensor.matmul(
                            pso,
                            lo_t[g][:, j, ts(m, 128)],
                            wo_t[:, 4 * g + j, ts(e4, 512)],
                            start=(j == 0),
                            stop=(j == 3),
                        )
                    if g == 0:
                        nc.vector.tensor_copy(o_acc[:, t, :], pso)
                    elif g < HPC - 1:
                        nc.vector.tensor_tensor(o_acc[:, t, :], pso, o_acc[:, t, :], OP.add)
                    else:
                        ob = dtmp.tile([128, 512], f32, tag="ob", name="ob")
                        nc.vector.tensor_tensor(ob, pso, o_acc[:, t, :], OP.add)
                        for p4 in range(4):
                            eng = nc.sync if p4 < 2 else nc.scalar
                            eng.dma_start(
                                out.ap()[m * 128 + p4 * 32:m * 128 + (p4 + 1) * 32,
                                         ts(e4, 512)],
                                ob[ts(p4, 32)],
                            )

                # o_proj work units [group, tile-range] pop into the PE stream
                # at i-chunk boundaries once their collective's data is safely
                # landed (cc + lo stage-in take ~13us after head g ends)
                oproj_units = []
                pending_norm = [None]

                def flush_norm():
                    if pending_norm[0] is not None:
                        pending_norm[0]()
                        pending_norm[0] = None

                def pop_units(k):
                    for _ in range(k):
                        if oproj_units:
                            g, t0, t1 = oproj_units.pop(0)
                            for t in range(t0, t1):
                                emit_oproj_tile(g, t)

                for h in range(HPC):
                    kt_h = k_t if h % 2 == 0 else k2_t
                    qm = q_t[h // 2]
                    if h in (0, 1):
                        # bulk w_o stream split across heads 0/1: halves the
                        # queue flood so cc0's staging and lo0 land in time
                        # for group 0's first pop; the scalar queue stays
                        # clear for exp
                        for k in range(8 * h, 8 * h + 8):
                            nc.sync.dma_start(wo_t[:, k, :], woT.ap()[ts(k, 128), :])
                    for i in range(NQ):
                        nj = 4 * i + 4
                        pr = probs_pool.tile([128, NB, 512], bf16, tag="pr")
                        po = psO.tile([128, 512], f32, tag="ps2", name="po")[0:D + 1, :]
                        # chunk j-blocks (pairs of full-width blocks share one
                        # psum tile + exp call), then emit P@V lagged a few
                        # chunks behind the scores so PE never stalls on exp
                        chunks = []
                        j = 0
                        while j < nj:
                            if j - 4 * i < -1:
                                chunks.append([j, j + 1])
                                j += 2
                            else:
                                chunks.append([j])
                                j += 1

                        def emit_scores(js, kt_h=kt_h, qm=qm, i=i, pr=pr):
                            if len(js) == 2:
                                pss = psS.tile([128, 2, 512], f32, tag="ps1", name="pss")
                                for u in range(2):
                                    nc.tensor.matmul(
                                        pss[:, u, :],
                                        kt_h[:, ts(js[u], 128)],
                                        qm[:, ts(i, 512)],
                                        start=True, stop=True,
                                    )
                                nc.scalar.activation(
                                    pr[:, js[0]:js[0] + 2, :], pss, AF.Exp, scale=SCALE
                                )
                            else:
                                jj = js[0]
                                r = jj - 4 * i
                                off = max(0, r) * 128
                                pss = psS.tile([128, 2, 512], f32, tag="ps1", name="pss")
                                nc.tensor.matmul(
                                    pss[:, 0, off:512],
                                    kt_h[:, ts(jj, 128)],
                                    qm[:, i * 512 + off:(i + 1) * 512],
                                    start=True, stop=True,
                                )
                                nc.scalar.activation(
                                    pr[:, jj, off:512], pss[:, 0, off:512],
                                    AF.Exp, scale=SCALE,
                                )
                                if r >= 0:  # block overlapping the causal diagonal
                                    nc.vector.tensor_tensor(
                                        pr[:, jj, off:off + 128],
                                        pr[:, jj, off:off + 128],
                                        tri_t, OP.mult,
                                    )

                        def emit_pv(js, i=i, pr=pr, po=po, nj=nj):
                            for jj in js:
                                off = max(0, jj - 4 * i) * 128
                                nc.tensor.matmul(
                                    po[:, off:512], vaug_t[:, jj, :], pr[:, jj, off:512],
                                    start=(jj == 0), stop=(jj == nj - 1),
                                )

                        LAG = 3
                        for ci, ch in enumerate(chunks):
                            emit_scores(ch)
                            if ci == min(1, len(chunks) - 1):
                                # previous i's normalize chain lands here so
                                # its PE step never waits on the DVE chain
                                flush_norm()
                                if i in (0, 1, 3):
                                    pop_units(1)
                            if ci >= LAG:
                                emit_pv(chunks[ci - LAG])
                        for ci in range(max(0, len(chunks) - LAG), len(chunks)):
                            emit_pv(chunks[ci])

                        def norm(h=h, i=i, po=po):
                            # oT[f, q] = po[f, q] / den[q]; den row broadcast
                            # across partitions via a K=1 ones matmul + 1/x on DVE
                            dbc = btmp.tile([D + 1, 512], bf16, tag="dbc", name="dbc")
                            nc.vector.tensor_copy(dbc[D:D + 1, :], po[D:D + 1, :])
                            pb = psD.tile([128, 512], f32, tag="ps3", name="pb")
                            nc.tensor.matmul(
                                pb[0:D, :], ones_t[D:D + 1, 0:D], dbc[D:D + 1, :],
                                start=True, stop=True,
                            )
                            rbs = btmp.tile([D, 512], f32, tag="rbs", name="rbs")
                            nc.vector.reciprocal_approx_fast(out=rbs, in_=pb[0:D, :])
                            oth = btmp.tile([D, 512], bf16, tag="oth", name="oth")
                            nc.vector.tensor_tensor(oth, po[0:D, :], rbs, OP.mult)
                            for half in range(2):
                                dd = nc.sync.dma_start(
                                    a2a_in[h].ap()[2 * i + half, :, :],
                                    oth[:, ts(half, 256)],
                                )
                                a2a_dmas[h].append(dd)
                        pending_norm[0] = norm
                        if i == 2 and h in (1, 2):
                            emit_oproj_load(h - 1)
                        if i == 1 and h == 3:
                            emit_oproj_load(2)
                        if i == 2 and h == 3:
                            oproj_units.extend([(2, 0, 4), (2, 4, 8)])
                    flush_norm()
                    _emit_cc(h)
                    # previous head's o_proj: 4 tiles now (PE covers the
                    # freshly-emitted collective), 2+2 early next head
                    if 1 <= h < 3:
                        oproj_units.extend(
                            [(h - 1, 0, 4), (h - 1, 4, 6), (h - 1, 6, 8)]
                        )
                    if h >= 2:
                        pop_units(1)

                # final group: head 3 + output assembly, double-buffered
                # across two psum pools so the adds never stall the PE
                g = HPC - 1
                emit_oproj_load(g, dual=True)
                pop_units(len(oproj_units))
                for t in range(8):
                    emit_oproj_tile(g, t, alt=True)

    nc.compile()
    return nc


def _get_nc():
    if "nc" not in _CACHE:
        _CACHE["nc"] = _build()
    return _CACHE["nc"]


def _host_prep(hidden_states, positions, w_qkv, w_o):
    bf16 = ml_dtypes.bfloat16
    hTb = np.ascontiguousarray(hidden_states.astype(np.float32).T).astype(bf16)
    # pretile to [n, p, ko, s] so each chunk is contiguous per partition
    hTt = np.ascontiguousarray(
        hTb.reshape(HID // 128, 128, T // 512, 512).transpose(2, 1, 0, 3)
    )
    woTb = np.ascontiguousarray(w_o.astype(np.float32).T).astype(bf16)
    # o_proj contraction order matches the per-head AllToAll staging:
    # group h holds head h of every source core (src-major within group)
    srcs = np.arange(NCORES)[:, None]
    dd_ = np.arange(D)[None, :]
    idx = np.concatenate(
        [(srcs * FPC + h * D + dd_).reshape(-1) for h in range(HPC)]
    )
    woTb = np.ascontiguousarray(woTb[idx])

    inv = 1.0 / (ROPE_THETA ** (np.arange(0, D, 2, dtype=np.float32) / D))  # [32]
    ang = positions.astype(np.float32)[:, None] * inv[None, :]              # [T, 32]
    cos = np.cos(ang).T  # [32, T]
    sin = np.sin(ang).T
    p = np.arange(128)
    fr = (p % D) % (D // 2)
    sgn = np.where((p % D) < (D // 2), -1.0, 1.0).astype(np.float32)
    cosf = np.ascontiguousarray(cos[fr])                     # [128, T]
    sinf = np.ascontiguousarray(sin[fr] * sgn[:, None])      # [128, T]
    # [n, 128, 512] chunk layout
    cosf = np.ascontiguousarray(cosf.reshape(128, NQ := T // 512, 512).transpose(1, 0, 2))
    sinf = np.ascontiguousarray(sinf.reshape(128, NQ, 512).transpose(1, 0, 2))

    partner = np.where((p % D) < (D // 2), p + D // 2, p - D // 2)
    perm = np.zeros((128, 128), dtype=np.float32)
    perm[p, partner] = 1.0
    ident = np.eye(128, dtype=np.float32)
    tri = (np.arange(128)[None, :] >= np.arange(128)[:, None]).astype(np.float32)
    ones_m = np.ones((128, 128), dtype=np.float32)
    pcb = np.ascontiguousarray(
        np.stack([perm, ident, tri, ones_m], axis=1).astype(bf16)
    )  # [128, 4, 128]

    q_size = NH * D
    kv_size = NKV * D
    in_maps = []
    for c in range(NCORES):
        wq = w_qkv[c * FPC:(c + 1) * FPC]
        wk = w_qkv[q_size + c * D:q_size + (c + 1) * D]
        wv = w_qkv[q_size + kv_size + c * D:q_size + kv_size + (c + 1) * D]
        wTc = np.ascontiguousarray(
            np.concatenate([wq, wk, wv], axis=0).astype(np.float32).T
        ).astype(bf16)
        # [128, KO, 384]: per-partition contiguous across contraction chunks
        wTc = np.ascontiguousarray(
            wTc.reshape(HID // 128, 128, QKV_F).transpose(1, 0, 2)
        )
        in_maps.append(
            {
                "hT": hTt,
                "wT": wTc,
                "cosf": cosf,
                "sinf": sinf,
                "pc": pcb,
                "woT": woTb,
            }
        )
    return in_maps


def run(inputs, trace=False):
    """Run on 8 NeuronCores; returns (full_output, BassKernelResults)."""
    if trace:
        _ensure_trace_hooks()
    from concourse import bass_utils

    if trace:
        bass_utils.upload_artifacts = lambda tmpdir: tmpdir
    nc = _get_nc()
    in_maps = _host_prep(
        np.asarray(inputs["hidden_states"]),
        np.asarray(inputs["positions"]),
        np.asarray(inputs["w_qkv"]),
        np.asarray(inputs["w_o"]),
    )
    res = bass_utils.run_bass_kernel_spmd(
        nc, in_maps, core_ids=list(range(NCORES)), trace=trace
    )
    full = np.concatenate(
        [res.results[c]["out"] for c in range(NCORES)], axis=0
    ).astype(np.float32)
    return full, res


def kernel(**inputs) -> np.ndarray:
    trace = bool(os.environ.get("KERNEL_TRACE"))
    full, _ = run(inputs, trace=trace)
    return full
